# revision 2
# baseline (speedup 1.0000x reference)
"""Bahdanau-attention kernel for trn2, 8-core data-parallel over batch.

Per core: 32 batches. Per batch b:
  1. SWDGE cast-DMA load enc[b] f32->bf16, natural layout [196, 2048]
     (two tiles: [128, 2048] + [80(68 valid), 2048]).
  2. xbar DMA-transpose -> encT [128, 16, 208] (e-chunk et holds
     e = et*128 + p; cols 0:196 valid l).
  3. PE: eT[a,l] = sum_e W_enc[e,a] * encT[e,l]  (64 bf16 matmuls,
     4 a-chunks x 16 e-chunks, psum [128, 196]).
  4. ACT: hT = relu(eT + D[b]) fused via activation bias (D = dec@W_dec
     + b_dec + b_enc precomputed once on PE), write bf16 SBUF.
  5. PE: att[1, 196] = W_full.T @ hT (4 matmuls, accumulate).
  6. ACT: w = exp(att) with fused accum_out -> Z.  (max-subtraction is
     skipped: att is O(1) by construction; b_full cancels in softmax.)
  7. DVE: rz = 1/Z; alpha = w * rz (bf16 + f32 copies).
  8. xbar-transpose alpha [1,196] -> alphaT [l-partition, 1] (2 calls).
  9. PE: ctx[1, 2048] = alphaT.T @ enc_nat (8 matmuls N=512, K=l).
 10. DVE/ACT: psum->sbuf copies; HWDGE stores of ctx, alpha.
"""

import os
import numpy as np

B, L, ENC, DEC, ATT = 256, 196, 2048, 512, 512
N_CORES = 8
BPC = B // N_CORES          # batches per core
LA, LB = 128, L - 128       # l-chunk split (68 valid in chunk B)
LBP = 80                    # chunk B padded to multiple of 16 for xbar
KT = ENC // 128             # 16 e-chunks
AT = ATT // 128             # 4 a-chunks
DT = DEC // 128             # 4 dec-chunks

_cache = {}


def _build(nb):
    import concourse.bass as bass  # noqa: F401
    import concourse.mybir as mybir
    import concourse.tile as tile
    from concourse import bacc

    f32 = mybir.dt.float32
    bf16 = mybir.dt.bfloat16
    Relu = mybir.ActivationFunctionType.Relu
    Exp = mybir.ActivationFunctionType.Exp

    nc = bacc.Bacc("TRN2", target_bir_lowering=False, debug=False,
                   num_devices=N_CORES)

    enc_d = nc.dram_tensor("enc", (BPC, L, ENC), f32, kind="ExternalInput")
    dec_d = nc.dram_tensor("dec", (BPC, DEC), f32, kind="ExternalInput")
    W_enc_d = nc.dram_tensor("W_enc", (ENC, ATT), f32, kind="ExternalInput")
    b_enc_d = nc.dram_tensor("b_enc", (1, ATT), f32, kind="ExternalInput")
    W_dec_d = nc.dram_tensor("W_dec", (DEC, ATT), f32, kind="ExternalInput")
    b_dec_d = nc.dram_tensor("b_dec", (1, ATT), f32, kind="ExternalInput")
    W_full_d = nc.dram_tensor("W_full", (ATT, 1), f32, kind="ExternalInput")
    ctx_d = nc.dram_tensor("ctx", (BPC, ENC), f32, kind="ExternalOutput")
    alpha_d = nc.dram_tensor("alpha", (BPC, L), f32, kind="ExternalOutput")

    with tile.TileContext(nc) as tc:
        with (
            tc.tile_pool(name="singles", bufs=1) as singles,
            tc.tile_pool(name="nat", bufs=3) as natp,
            tc.tile_pool(name="enct", bufs=2) as enctp,
            tc.tile_pool(name="ht", bufs=2) as htp,
            tc.tile_pool(name="small", bufs=4) as smallp,
            tc.tile_pool(name="pse", bufs=3, space="PSUM") as pse,
            tc.tile_pool(name="psatt", bufs=1, space="PSUM") as psatt,
            tc.tile_pool(name="psctx", bufs=4, space="PSUM") as psctx,
        ):
            # ---- constants / weights (once) ----
            W_enc_sb = singles.tile([128, KT, ATT], bf16)
            for kt in range(KT):
                nc.gpsimd.dma_start(out=W_enc_sb[:, kt, :],
                                    in_=W_enc_d[kt * 128:(kt + 1) * 128, :])
            Wd_sb = singles.tile([128, DT, ATT], bf16)
            for dt in range(DT):
                nc.gpsimd.dma_start(out=Wd_sb[:, dt, :],
                                    in_=W_dec_d[dt * 128:(dt + 1) * 128, :])
            Wf_sb = singles.tile([128, AT], bf16)
            nc.gpsimd.dma_start(
                out=Wf_sb,
                in_=W_full_d.rearrange("(at p) o -> p (at o)", p=128))

            bias_f = singles.tile([1, ATT], f32)
            tmp_bd = singles.tile([1, ATT], f32)
            nc.sync.dma_start(out=bias_f, in_=b_enc_d[:, :])
            nc.sync.dma_start(out=tmp_bd, in_=b_dec_d[:, :])
            nc.vector.tensor_add(out=bias_f, in0=bias_f, in1=tmp_bd)
            bias_bf = singles.tile([1, ATT], bf16)
            nc.vector.tensor_copy(out=bias_bf, in_=bias_f)
            ones_bf = singles.tile([1, BPC], bf16)
            nc.vector.memset(ones_bf, 1.0)

            # dec -> decT via cast-load + xbar  (d = dt*128 + p)
            dec_bf = singles.tile([BPC, DEC], bf16)
            nc.gpsimd.dma_start(out=dec_bf, in_=dec_d[:, :])
            decT = singles.tile([128, DT, BPC], bf16)
            nc.sync.dma_start(out=decT, in_=dec_bf, transpose=True)

            # D[a, b] = sum_d W_dec[d, a] dec[b, d] + (b_dec + b_enc)[a]
            D_sb = singles.tile([128, AT, BPC], f32)
            for at in range(AT):
                ps = pse.tile([128, BPC], f32, tag="ps_e")
                for dt in range(DT):
                    nc.tensor.matmul(ps,
                                     Wd_sb[:, dt, at * 128:(at + 1) * 128],
                                     decT[:, dt, :],
                                     start=(dt == 0), stop=False)
                nc.tensor.matmul(ps, bias_bf[0:1, at * 128:(at + 1) * 128],
                                 ones_bf, start=False, stop=True)
                nc.vector.tensor_copy(out=D_sb[:, at, :], in_=ps)

            # ---- main batch loop ----
            for b in range(nb):
                nat_a = natp.tile([128, ENC], bf16, tag="nat_a")
                nat_b = natp.tile([LBP, ENC], bf16, tag="nat_b")
                nc.gpsimd.dma_start(out=nat_a, in_=enc_d[b, 0:LA, :])
                nc.gpsimd.dma_start(out=nat_b[0:LB, :], in_=enc_d[b, LA:L, :])

                enct = enctp.tile([128, KT, 208], bf16)
                nc.sync.dma_start(out=enct[:, :, 0:128], in_=nat_a,
                                  transpose=True)
                nc.sync.dma_start(out=enct[:, :, 128:208], in_=nat_b,
                                  transpose=True)

                ht = htp.tile([128, AT, L], bf16)
                for at in range(AT):
                    ps_e = pse.tile([128, L], f32, tag="ps_e")
                    for kt in range(KT):
                        nc.tensor.matmul(
                            ps_e,
                            W_enc_sb[:, kt, at * 128:(at + 1) * 128],
                            enct[:, kt, 0:L],
                            start=(kt == 0), stop=(kt == KT - 1))
                    nc.scalar.activation(out=ht[:, at, :], in_=ps_e,
                                         func=Relu,
                                         bias=D_sb[:, at, b:b + 1], scale=1.0)

                ps_att = psatt.tile([1, L], f32)
                for at in range(AT):
                    nc.tensor.matmul(ps_att, Wf_sb[:, at:at + 1],
                                     ht[:, at, :],
                                     start=(at == 0), stop=(at == AT - 1))

                w_sb = smallp.tile([1, L], f32, tag="w")
                z_sb = smallp.tile([1, 1], f32, tag="z")
                nc.scalar.activation(out=w_sb, in_=ps_att[0:1, :], func=Exp,
                                     accum_out=z_sb)
                rz = smallp.tile([1, 1], f32, tag="rz")
                nc.vector.reciprocal(out=rz, in_=z_sb)

                alpha_pad = smallp.tile([16, 256], bf16, tag="apad")
                nc.vector.tensor_scalar_mul(out=alpha_pad[0:1, 0:L],
                                            in0=w_sb, scalar1=rz)
                alpha_f = smallp.tile([1, L], f32, tag="af")
                nc.scalar.mul(out=alpha_f, in_=w_sb, mul=rz)
                nc.sync.dma_start(out=alpha_d[b:b + 1, :], in_=alpha_f)

                aT_a = smallp.tile([128, 16], bf16, tag="aTa")
                aT_b = smallp.tile([128, 16], bf16, tag="aTb")
                nc.sync.dma_start(out=aT_a, in_=alpha_pad[:, 0:128],
                                  transpose=True)
                nc.sync.dma_start(out=aT_b, in_=alpha_pad[:, 128:256],
                                  transpose=True)

                ctx_sb = smallp.tile([1, ENC], f32, tag="ctx")
                for q in range(4):
                    ps_c = psctx.tile([1, 512], f32, tag="ps_c")
                    nc.tensor.matmul(ps_c, aT_a[:, 0:1],
                                     nat_a[:, q * 512:(q + 1) * 512],
                                     start=True, stop=False)
                    nc.tensor.matmul(ps_c, aT_b[0:LB, 0:1],
                                     nat_b[0:LB, q * 512:(q + 1) * 512],
                                     start=False, stop=True)
                    if q < 2:
                        nc.vector.tensor_copy(
                            out=ctx_sb[0:1, q * 512:(q + 1) * 512], in_=ps_c)
                    else:
                        nc.scalar.copy(
                            out=ctx_sb[0:1, q * 512:(q + 1) * 512], in_=ps_c)
                nc.sync.dma_start(out=ctx_d[b:b + 1, :], in_=ctx_sb)

    nc.compile()
    return nc


def _get(nb):
    if nb not in _cache:
        _cache[nb] = _build(nb)
    return _cache[nb]


def kernel(enc, dec, W_enc, b_enc, W_dec, b_dec, W_full, b_full=None,
           _nb=None):
    from concourse.bass_utils import run_bass_kernel_spmd

    nb = BPC if _nb is None else _nb
    nc = _get(nb)

    enc = np.ascontiguousarray(np.asarray(enc, dtype=np.float32))
    dec = np.ascontiguousarray(np.asarray(dec, dtype=np.float32))
    shared = {
        "W_enc": np.ascontiguousarray(np.asarray(W_enc, np.float32)),
        "b_enc": np.asarray(b_enc, np.float32).reshape(1, ATT),
        "W_dec": np.ascontiguousarray(np.asarray(W_dec, np.float32)),
        "b_dec": np.asarray(b_dec, np.float32).reshape(1, ATT),
        "W_full": np.ascontiguousarray(np.asarray(W_full, np.float32)),
    }
    in_maps = []
    for c in range(N_CORES):
        m = dict(shared)
        m["enc"] = enc[c * BPC:(c + 1) * BPC]
        m["dec"] = dec[c * BPC:(c + 1) * BPC]
        in_maps.append(m)

    res = run_bass_kernel_spmd(nc, in_maps, core_ids=list(range(N_CORES)),
                               trace=bool(int(os.environ.get("KERNEL_TRACE",
                                                             "0"))))
    ctx = np.concatenate([res.results[c]["ctx"] for c in range(N_CORES)], 0)
    alpha = np.concatenate([res.results[c]["alpha"]
                            for c in range(N_CORES)], 0)
    kernel.last_results = res
    return ctx.astype(np.float32), alpha.reshape(B, L, 1).astype(np.float32)
